# revision 27
# baseline (speedup 1.0000x reference)
"""Trainium2 Bass kernel for a 2-layer LLaMA-style decoder with per-layer
memory K/V prefix (tokenmix2 Decoder), tensor-parallel over 8 NeuronCores.

Sharding: heads (32 -> 4/core), FFN intermediate (8192 -> 1024/core),
vocab (8192 -> 1024/core).  Two AllReduces per layer site (attention out,
FFN out), one per 512-token sequence chunk ([D, 512] fp16 = 4MB each).

The whole kernel is a 2-stage sequence-chunk pipeline: each chunk's
dependency chain is emitted contiguously, so while chunk c's AllReduce is
in flight the engines run chunk 1-c's compute.

Round-1 perf structure vs the original baseline:
- AR-dependent loads (delta reads from arout) are issued from gpsimd
  (SWDGE) so the sync-engine HWDGE queue never head-of-line blocks on an
  AllReduce-completion semaphore (this froze ALL weight DMA issue for the
  whole AR window in the baseline).
- A 1MB warmup AllReduce at kernel start absorbs first-collective cost.
- rms scaling applied on the OUTPUT side for attention + lm-head sites
  (ln1/normw folded into weights on the host; 1/rms applied via the rope
  raw-evacuation / V-evacuation per-partition scale / logit evacuation).
- rsqrt and softmax reciprocal via exp(-ln(x)...) on the scalar engine
  (replaces the 3.3us DVE iterative reciprocal).
- attention scores pipelined 2 key-tiles ahead of the PV accumulation;
  causal mask is multiplicative 0/1 on exp(scores) in SBUF.
- rope matmuls emitted in 2-head groups behind the next projection
  stream so the PSUM-evacuation latency is hidden; V projection halves
  interleaved with K pairs so the second V-weight DMA is covered.
- all weight DMAs fully contiguous in DRAM; evacuation stores issued
  from the scalar-engine HWDGE queue (separate ring from weight loads).
"""
import sys

sys.path.insert(0, "/opt/trn_rl_repo")

import numpy as np
import ml_dtypes

import concourse.bass as bass
import concourse.mybir as mybir
import concourse.tile as tile
from concourse import bacc
from concourse.bass_utils import run_bass_kernel_spmd

BF = np.float16

# model dims
L, D, H, DH, F, V = 2, 4096, 32, 128, 8192, 8192
B, S, M = 1, 1024, 512
T = M + S                      # 1536 total key positions
EPS = 1e-5
ROPE_BASE = 10000.0
SCALE = float(DH) ** -0.5
LN64 = float(np.log(64.0))

# per-core shards
NCORES = 8
HL = H // NCORES               # 4 local heads
DL = HL * DH                   # 512 local head dims
FL = F // NCORES               # 1024 local ffn
VL = V // NCORES               # 1024 local vocab
C = D // 128                   # 32 contraction tiles
NTT = T // 128                 # 12 key tiles
NMT = M // 128                 # 4 memory key tiles
NST = S // 128                 # 8 query tiles
NCH = 2                        # sequence chunks (512 each)
SC = S // NCH                  # 512

dt = mybir.dt
AF = mybir.ActivationFunctionType
ALU = mybir.AluOpType


def build_module():
    nc = bacc.Bacc("TRN2", target_bir_lowering=False, debug=False,
                   num_devices=NCORES)

    # const APs for activation scale/bias floats
    for v in (EPS, SCALE, 1.0 / D, LN64):
        t = nc.alloc_sbuf_tensor(f"cst_{v}", [128, 1], dt.float32)
        nc.gpsimd.memset(t.ap(), v)
        nc.const_aps.aps[(dt.float32, v)] = t.ap()

    # ---- kernel I/O ----
    h0T = nc.dram_tensor("h0T", [D, S], dt.float16, kind="ExternalInput")
    memT = nc.dram_tensor("memT", [L, 128, C, M], dt.float16, kind="ExternalInput")
    wqkT = nc.dram_tensor("wqkT", [L, 2, 4, 128, C, 128], dt.float16, kind="ExternalInput")
    wvT = nc.dram_tensor("wvT", [L, 2, 128, C, 256], dt.float16, kind="ExternalInput")
    wmkT = nc.dram_tensor("wmkT", [L, 4, 128, C, 128], dt.float16, kind="ExternalInput")
    wmvT = nc.dram_tensor("wmvT", [L, 2, 128, C, 256], dt.float16, kind="ExternalInput")
    woT = nc.dram_tensor("woT", [L, 2, 128, HL, 2048], dt.float16, kind="ExternalInput")
    wguT = nc.dram_tensor("wguT", [L, 2, 8, 128, C, 128], dt.float16, kind="ExternalInput")
    wdT = nc.dram_tensor("wdT", [L, 4, 128, 8, 1024], dt.float16, kind="ExternalInput")
    lmT = nc.dram_tensor("lmT", [8, 128, C, 128], dt.float16, kind="ExternalInput")
    kcs = nc.dram_tensor("kcs", [128, 2, T], dt.float16, kind="ExternalInput")
    rmat_i = nc.dram_tensor("rmat", [128, 128], dt.float16, kind="ExternalInput")
    tmask = nc.dram_tensor("tmask", [128, 896], dt.float16, kind="ExternalInput")
    lnw = nc.dram_tensor("lnw", [128, 5, C], dt.float32, kind="ExternalInput")
    logitsT = nc.dram_tensor("logitsT", [VL, S], dt.float32, kind="ExternalOutput")

    with tile.TileContext(nc) as tc:
        with tc.tile_pool(name="sb", bufs=1) as sb, \
             tc.tile_pool(name="ps", bufs=1, space="PSUM") as ps, \
             tc.tile_pool(name="dr", bufs=1, space="DRAM") as dr:

            # ---- internal DRAM ----
            hdr = [dr.tile([D, S], dt.float16, tag=f"h{i}", bufs=1, name=f"h{i}")
                   for i in range(3)]           # h after resid 1..3
            arin = [[dr.tile([D, SC], dt.float16, tag=f"ai{i}{ch}",
                             bufs=1, name=f"ai{i}{ch}") for ch in range(NCH)]
                    for i in range(2 * L)]
            arout = [[dr.tile([D, SC], dt.float16, tag=f"ao{i}{ch}",
                              bufs=1, addr_space="Shared",
                              name=f"ao{i}{ch}") for ch in range(NCH)]
                     for i in range(2 * L)]
            mkTd = [dr.tile([128, HL, M], dt.float16, tag=f"mk{l}", bufs=1,
                            name=f"mk{l}") for l in range(L)]
            mvd = [dr.tile([128, HL, NMT, DH], dt.float16, tag=f"mv{l}", bufs=1,
                           name=f"mv{l}") for l in range(L)]
            war_i = dr.tile([128, 4096], dt.float16, tag="wri", bufs=1,
                            name="war_i")
            war_o = dr.tile([128, 4096], dt.float16, tag="wro", bufs=1,
                            addr_space="Shared", name="war_o")

            def wp_tile(name):
                return sb.tile([128, C, 128], dt.float16, tag="wp", bufs=3,
                               name=name)

            # ---- global constants in SBUF ----
            kc = sb.tile([128, 2, S], dt.float16, tag="kc", bufs=1, name="kc")
            nc.sync.dma_start(kc[:], kcs[:, :, M:])
            # keys and queries of the S-part sit at the same positions
            qc = kc
            rmat = sb.tile([128, 128], dt.float16, tag="rm", bufs=1, name="rmat")
            nc.sync.dma_start(rmat[:], rmat_i[:])
            mask = sb.tile([128, 896], dt.float16, tag="msk", bufs=1, name="mask")
            nc.sync.dma_start(mask[:], tmask[:])
            lns = sb.tile([128, 5, C], dt.float32, tag="ln", bufs=1, name="lns")
            nc.sync.dma_start(lns[:], lnw[:])
            ones_bf = sb.tile([128, 1], dt.float16, tag="o1", bufs=1, name="ones_bf")
            nc.vector.memset(ones_bf[:], 1.0)
            ones_row = sb.tile([1, 128], dt.float16, tag="o2", bufs=1, name="ones_row")
            nc.vector.memset(ones_row[:], 1.0)
            inv64_row = sb.tile([1, 128], dt.float16, tag="o3", bufs=1, name="inv64_row")
            nc.vector.memset(inv64_row[:], 1.0 / 64.0)

            def mm_ps(name):
                return ps.tile([128, 512], dt.float32, tag="mm", bufs=4, name=name)

            def aux_ps(name):
                return ps.tile([1, 512], dt.float32, tag="aux", bufs=1, name=name)

            def evh(name):
                return sb.tile([128, 512], dt.float16, tag="evh", bufs=4, name=name)

            # ---- rope pieces (split so the rmat matmuls can be emitted
            # behind the next head's projection stream).  If bc_sb is
            # given, the 1/rms row scale is folded into the raw psum
            # evacuation (DVE) -- rope(x*r) == rope(x)*r columnwise. ----
            def rope_start(raw_ps, w=512, bc_sb=None):
                raw_bf = sb.tile([128, 512], dt.float16, tag="rraw", bufs=2,
                                 name="raw_bf")
                if bc_sb is None:
                    nc.scalar.activation(raw_bf[:, :w], raw_ps, AF.Copy)
                else:
                    nc.vector.tensor_tensor(raw_bf[:, :w], raw_ps,
                                            bc_sb[:, :w], ALU.mult)
                return raw_bf

            def rope_mm(raw_bf, w=512):
                r_ps = mm_ps("r_ps")
                nc.tensor.matmul(r_ps[:, :w], rmat[:], raw_bf[:, :w],
                                 start=True, stop=True)
                return r_ps

            def rope_finish(raw_bf, r_ps, cos_ap, sin_ap, out_ap, w=512):
                m1 = sb.tile([128, 512], dt.float16, tag="rt", bufs=2, name="m1")
                nc.gpsimd.tensor_tensor(m1[:, :w], raw_bf[:, :w], cos_ap,
                                        ALU.mult)
                m2 = sb.tile([128, 512], dt.float16, tag="rt2", bufs=2, name="m2")
                nc.vector.tensor_tensor(m2[:, :w], r_ps[:, :w], sin_ap, ALU.mult)
                nc.vector.tensor_tensor(out_ap, m1[:, :w], m2[:, :w], ALU.add)

            # =========================================================
            # memory K/V projections for one layer -> DRAM (M in halves
            # of 256 to halve the mem_sb SBUF footprint)
            # =========================================================
            def mem_proj(l, mh):
                msl = slice(256 * mh, 256 * (mh + 1))
                mem_h = sb.tile([128, C, 256], dt.float16, tag="memh",
                                bufs=1, name=f"mem{l}{mh}")
                nc.sync.dma_start(mem_h[:, :C // 2, :],
                                  memT[l][:, :C // 2, msl])
                nc.sync.dma_start(mem_h[:, C // 2:, :],
                                  memT[l][:, C // 2:, msl])
                # rope tables for memory positions (kc only holds S-part)
                mtab = sb.tile([128, 4, 512], dt.float16, tag="dl", bufs=2,
                               name=f"mtab{l}{mh}")
                nc.sync.dma_start(mtab[:, 0, :], kcs[:, :, msl])
                mcos, msin = mtab[:, 0, 0:256], mtab[:, 0, 256:512]
                def mk_pair(pair):
                    accs = []
                    for dd in range(2):
                        d = 2 * pair + dd
                        wmk = wp_tile(f"wmk{l}{d}{mh}")
                        nc.sync.dma_start(wmk[:], wmkT[l, d])
                        acc = mm_ps(f"mk{l}{d}{mh}")
                        for c in range(C):
                            nc.tensor.matmul(acc[:, :256], wmk[:, c, :],
                                             mem_h[:, c, :], start=(c == 0),
                                             stop=(c == C - 1))
                        accs.append((d, acc[:, :256]))
                    raws = [(d, rope_start(acc, w=256)) for d, acc in accs]
                    rps = [(d, raw, rope_mm(raw, w=256)) for d, raw in raws]
                    for d, raw, rp in rps:
                        mko = sb.tile([128, 256], dt.float16, tag="mko",
                                      bufs=2, name="mko")
                        rope_finish(raw, rp, mcos, msin,
                                    mko[:], w=256)
                        nc.scalar.dma_start(mkTd[l][:, d, msl], mko[:])

                def mv_half(half):
                    wmv = sb.tile([128, C, 256], dt.float16, tag="wv16",
                                  bufs=1, name=f"wmv{l}{mh}{half}")
                    nc.sync.dma_start(wmv[:], wmvT[l, half])
                    for mi in range(2):
                        t = 2 * mh + mi
                        acc = mm_ps(f"mv{l}{mh}{half}{mi}")
                        for c in range(C):
                            nc.tensor.matmul(
                                acc[:, :256],
                                mem_h[:, c, 128 * mi:128 * (mi + 1)],
                                wmv[:, c, :], start=(c == 0),
                                stop=(c == C - 1))
                        mvo = sb.tile([128, 256], dt.float16, tag="mvo",
                                      bufs=2, name="mvo")
                        nc.scalar.activation(mvo[:], acc[:, :256], AF.Copy)
                        nc.scalar.dma_start(
                            mvd[l][:, 2 * half:2 * half + 2, t, :], mvo[:])

                # interleave so the second wmv DMA hides behind mk compute
                mk_pair(0)
                mv_half(0)
                mk_pair(1)
                mv_half(1)

            # =========================================================
            # rms for ONE chunk.
            # mode "scale": scale xb in place (ffn sites).
            # mode "bc": no input scaling -- returns (bc_sb fp16 [128,SC]
            #   row-broadcast of 1/rms, rs_col f32 [128,4] column form).
            #   ln / final-norm weights are folded into the weights on
            #   the host for these sites.
            # =========================================================
            def rms_chunk(ch, h_src, delta, h_dst, ln_idx, xb, name, mode,
                          order_dep=None, preloaded=False):
                ssq = aux_ps(f"ssq_{name}{ch}")
                if not preloaded:
                    hv = h_src.rearrange("(c p) s -> p c s", p=128)
                    # prefetch the full h chunk first (not AR-gated)
                    for cq in range(C // 2):
                        csl = slice(2 * cq, 2 * cq + 2)
                        nc.sync.dma_start(xb[:, csl, :],
                                          hv[:, csl, SC * ch:SC * (ch + 1)])
                for cq in range(C // 4):
                    csl = slice(4 * cq, 4 * cq + 4)
                    if delta is not None:
                        dtl = sb.tile([128, 4, 512], dt.float16, tag="dl",
                                      bufs=2, name="dtl")
                        if order_dep is not None and cq < 2:
                            # dummy write that READS the previous chunk's
                            # last vector output: forces the scheduler to
                            # place this whole AR-gated chain after all of
                            # the previous chunk's engine work, so the
                            # in-order queues never head-of-line block on
                            # the AllReduce semaphore.
                            nc.vector.tensor_tensor(dtl[:, 0, :], order_dep,
                                                    order_dep, ALU.mult)
                        # gpsimd (SWDGE) so the AR-completion wait never
                        # blocks the sync HWDGE queue
                        nc.gpsimd.dma_start(
                            dtl[:],
                            delta.rearrange("(c p) s -> p c s", p=128)[:, csl, :])
                        for ci in range(4):
                            nc.vector.tensor_tensor(xb[:, 4 * cq + ci, :],
                                                    xb[:, 4 * cq + ci, :],
                                                    dtl[:, ci, :], ALU.add)
                        if h_dst is not None:
                            nc.sync.dma_start(
                                h_dst.rearrange("(c p) s -> p c s", p=128)
                                [:, csl, SC * ch:SC * (ch + 1)],
                                xb[:, csl, :])
                    for ci in range(4):
                        c = 4 * cq + ci
                        hsq = sb.tile([128, 512], dt.float16, tag="hsq",
                                      bufs=2, name="hsq")
                        nc.vector.tensor_tensor(hsq[:], xb[:, c, :],
                                                xb[:, c, :], ALU.mult)
                        nc.tensor.matmul(ssq[:], ones_bf[:], hsq[:],
                                         start=(c == 0), stop=(c == C - 1))
                # 1/rms row: exp(-0.5 * ln(ssq/D + eps)) on ACT (fast;
                # avoids the 3.3us DVE iterative reciprocal)
                def emit_row():
                    lnr = sb.tile([1, 512], dt.float32, tag="row", bufs=1,
                                  name="lnr")
                    nc.scalar.activation(lnr[:], ssq[:], AF.Ln, bias=EPS,
                                         scale=1.0 / D)
                    rs = sb.tile([1, 512], dt.float16, tag="row2", bufs=2,
                                 name="rs")
                    nc.scalar.activation(rs[:], lnr[:], AF.Exp, scale=-0.5)
                    bc_ps = ps.tile([128, 512], dt.float32, tag="bc", bufs=2,
                                    name="bc")
                    nc.tensor.matmul(bc_ps[:], ones_row[:], rs[:], start=True,
                                     stop=True)
                    nc.scalar.activation(bc_sb[:], bc_ps[:], AF.Copy)
                    return rs
                if mode == "scale":
                    # ln2 is folded into Wg/Wu on the host, so the rescale
                    # is a plain 2-operand multiply (fast DVE mode)
                    bc_sb = sb.tile([128, 512], dt.float16, tag="bcb", bufs=2,
                                    name="bcs16")
                    emit_row()
                    for c in range(C):
                        nc.vector.tensor_tensor(
                            xb[:, c, :], xb[:, c, :], bc_sb[:], ALU.mult)
                    return None, None, None
                bc_sb = sb.tile([128, 512], dt.float16, tag="bcb", bufs=2,
                                name="bc_sb")
                rs_col = sb.tile([128, 4], dt.float32, tag="rscs", bufs=2,
                                 name="rs_col")

                def finish():
                    # deferred: emitted behind the first projection block
                    # so the ACT row latency never stalls the PE
                    rs = emit_row()
                    rsc = ps.tile([128, 4], dt.float32, tag="rsc", bufs=1,
                                  name="rsc")
                    for st in range(4):
                        nc.tensor.matmul(rsc[:, st:st + 1],
                                         rs[:, 128 * st:128 * (st + 1)],
                                         ones_row[:, 0:1], start=True,
                                         stop=True)
                    nc.scalar.activation(rs_col[:], rsc[:], AF.Copy)
                return bc_sb, rs_col, finish

            # =========================================================
            # attention pieces (KT/Vt persist for the layer)
            # =========================================================
            def attn_v_half(l, ch, half, xb, Vt, rs_col, finish_rms=None):
                wv = sb.tile([128, C, 256], dt.float16, tag="wv16",
                             bufs=1, name=f"wv{l}{ch}{half}")
                nc.sync.dma_start(wv[:], wvT[l, half])
                for sti in range(4):
                    st = 4 * ch + sti
                    acc = mm_ps(f"v{l}{half}{st}")
                    for c in range(C):
                        nc.tensor.matmul(
                            acc[:, :256],
                            xb[:, c, 128 * sti:128 * (sti + 1)],
                            wv[:, c, :], start=(c == 0), stop=(c == C - 1))
                    if sti == 0 and finish_rms is not None:
                        finish_rms()
                    # per-key 1/rms via per-partition ACT scale
                    nc.scalar.activation(Vt[:, 2 * half:2 * half + 2,
                                            NMT + st, :],
                                         acc[:, :256], AF.Copy,
                                         scale=rs_col[:, sti:sti + 1])

            def attn_k_pair(l, ch, pair, xb, KT, bc_sb):
                accs = []
                for dd in range(2):
                    d = 2 * pair + dd
                    wk = wp_tile(f"wk{l}{ch}{d}")
                    nc.sync.dma_start(wk[:], wqkT[l, 1, d])
                    acc = mm_ps(f"k{l}{d}{ch}")
                    for c in range(C):
                        nc.tensor.matmul(
                            acc[:], wk[:, c, :],
                            xb[:, c, :], start=(c == 0), stop=(c == C - 1))
                    accs.append((d, acc))
                raws = [(d, rope_start(acc, bc_sb=bc_sb)) for d, acc in accs]
                rps = [(d, raw, rope_mm(raw)) for d, raw in raws]
                for d, raw, rp in rps:
                    rope_finish(raw, rp,
                                kc[:, 0, SC * ch:SC * (ch + 1)],
                                kc[:, 1, SC * ch:SC * (ch + 1)],
                                KT[:, d, M + SC * ch:M + SC * (ch + 1)])

            def attn_q(l, ch, xb, qTc, bc_sb):
                for pair in range(2):
                    accs = []
                    for hh in range(2):
                        h = 2 * pair + hh
                        wqh = wp_tile(f"wq{l}{ch}{h}")
                        nc.sync.dma_start(wqh[:], wqkT[l, 0, h])
                        acc = mm_ps(f"q{l}{h}{ch}")
                        for c in range(C):
                            nc.tensor.matmul(
                                acc[:], wqh[:, c, :],
                                xb[:, c, :], start=(c == 0), stop=(c == C - 1))
                        accs.append((h, acc))
                    raws = [(h, rope_start(acc, bc_sb=bc_sb)) for h, acc in accs]
                    rps = [(h, raw, rope_mm(raw)) for h, raw in raws]
                    for h, raw, rp in rps:
                        rope_finish(raw, rp,
                                    qc[:, 0, SC * ch:SC * (ch + 1)],
                                    qc[:, 1, SC * ch:SC * (ch + 1)],
                                    qTc[:, h, :])

            def attn_S(l, sb_i, qTc, oTc, KT, Vt):
                ntt = NMT + 4 * (sb_i + 1)
                LA = 3
                pending = [None]

                def normalize_act():
                    # ACT row chain emitted right behind the head's last
                    # s-matmul, ahead of the next head's exp stream
                    h, o_ps, s_ps = pending[0]
                    lnr = sb.tile([1, 512], dt.float32, tag="row", bufs=1,
                                  name="lnr_s")
                    nc.scalar.activation(lnr[:], s_ps[:], AF.Ln)
                    rr = sb.tile([1, 512], dt.float16, tag="row2", bufs=2,
                                 name="rr")
                    nc.scalar.activation(rr[:], lnr[:], AF.Exp, scale=-1.0,
                                         bias=LN64)
                    pending[0] = (h, o_ps, rr)

                def normalize_mm():
                    h, o_ps, rr = pending[0]
                    bcp = ps.tile([128, 512], dt.float32, tag="bc", bufs=2,
                                  name="bca")
                    nc.tensor.matmul(bcp[:], inv64_row[:], rr[:],
                                     start=True, stop=True)
                    bcs = sb.tile([128, 512], dt.float32, tag="bcs",
                                  bufs=1, name="bcs")
                    nc.vector.tensor_copy(bcs[:], bcp[:])
                    nc.vector.tensor_tensor(oTc[:, h, :], o_ps[:], bcs[:],
                                            ALU.mult)
                    pending[0] = None

                for h in range(HL):
                    s_ps = aux_ps(f"s{l}{h}{sb_i}")
                    pts = {}

                    def emit_sc(tt, h=h):
                        sc_ps = mm_ps(f"sc{l}{h}{sb_i}{tt}")
                        nc.tensor.matmul(sc_ps[:],
                                         KT[:, h, 128 * tt:128 * (tt + 1)],
                                         qTc[:, h, :], start=True, stop=True)
                        pt = sb.tile([128, 512], dt.float16, tag="pt",
                                     bufs=4, name="pt")
                        nc.scalar.activation(pt[:], sc_ps[:], AF.Exp,
                                             scale=SCALE)
                        dtile = tt - ntt + 4      # >= 0 -> diagonal tile
                        if dtile >= 0:
                            off = 384 - 128 * dtile
                            nc.vector.tensor_tensor(
                                pt[:], pt[:],
                                mask[:, off:off + 512], ALU.mult)
                        pts[tt] = pt

                    if pending[0] is not None:
                        normalize_act()
                    for tt in range(min(LA, ntt)):
                        emit_sc(tt)
                    # previous head's broadcast-mm lands behind this
                    # head's warmup scores
                    if pending[0] is not None:
                        normalize_mm()
                    o_ps = mm_ps(f"o{l}{h}{sb_i}")
                    for tt in range(ntt):
                        pt = pts.pop(tt)
                        nc.tensor.matmul(o_ps[:], Vt[:, h, tt, :], pt[:],
                                         start=(tt == 0),
                                         stop=(tt == ntt - 1))
                        nc.tensor.matmul(s_ps[:], ones_bf[:], pt[:],
                                         start=(tt == 0),
                                         stop=(tt == ntt - 1))
                        if tt + LA < ntt:
                            emit_sc(tt + LA)
                    pending[0] = (h, o_ps, s_ps)
                normalize_act()
                normalize_mm()

            def attn_wo(l, ch, site, oTc):
                for half in range(2):
                  for doh in range(2):
                    wo = sb.tile([128, HL, 1024], dt.float16, tag="wp", bufs=3,
                                 name=f"wo{l}{ch}{half}{doh}")
                    nc.sync.dma_start(
                        wo[:], woT[l, half][:, :, 1024 * doh:1024 * (doh + 1)])
                    for do in range(8):
                        acc = mm_ps(f"wo{l}{half}{doh}{do}{ch}")
                        for hh in range(HL):
                            nc.tensor.matmul(
                                acc[:], wo[:, hh, 128 * do:128 * (do + 1)],
                                oTc[:, hh, :],
                                start=(hh == 0), stop=(hh == HL - 1))
                        ev = evh("woev")
                        # alternate evacuation engines so a single queue
                        # never paces the PSUM free rate
                        if do % 2 == 0:
                            nc.scalar.activation(ev[:], acc[:], AF.Copy)
                        else:
                            nc.vector.tensor_copy(ev[:], acc[:])
                        nc.sync.dma_start(
                            arin[site][ch]
                            .rearrange("(t p) s -> p t s", p=128)
                            [:, 16 * half + 8 * doh + do, :], ev[:])
                nc.gpsimd.collective_compute(
                    "AllReduce", ALU.add,
                    replica_groups=[list(range(NCORES))],
                    ins=[arin[site][ch][:]],
                    outs=[arout[site][ch][:]])

            # =========================================================
            # FFN pieces
            # =========================================================
            def ffn_gu(l, ch, xb, actTc):
                for fe in range(FL // 128):
                    wg = wp_tile(f"wg{l}{ch}{fe}")
                    nc.sync.dma_start(wg[:], wguT[l, 0, fe])
                    gs = sb.tile([128, 512], dt.float16, tag="gs", bufs=2,
                                 name="gs")
                    acc = mm_ps(f"g{l}{fe}{ch}")
                    for c in range(C):
                        nc.tensor.matmul(acc[:], wg[:, c, :],
                                         xb[:, c, :], start=(c == 0),
                                         stop=(c == C - 1))
                    nc.scalar.activation(gs[:], acc[:], AF.Silu)
                    wu = wp_tile(f"wu{l}{ch}{fe}")
                    nc.sync.dma_start(wu[:], wguT[l, 1, fe])
                    acc2 = mm_ps(f"u{l}{fe}{ch}")
                    for c in range(C):
                        nc.tensor.matmul(acc2[:], wu[:, c, :],
                                         xb[:, c, :], start=(c == 0),
                                         stop=(c == C - 1))
                    nc.vector.tensor_tensor(actTc[:, fe, :], acc2[:],
                                            gs[:], ALU.mult)

            def ffn_down(l, ch, site, actTc):
                for quarter in range(4):
                  for dh in range(2):
                    wd = sb.tile([128, FL // 128, 512], dt.float16, tag="wp",
                                 bufs=3, name=f"wd{l}{ch}{quarter}{dh}")
                    nc.sync.dma_start(
                        wd[:], wdT[l, quarter][:, :, 512 * dh:512 * (dh + 1)])
                    for do in range(4):
                        acc = mm_ps(f"wd{l}{quarter}{dh}{do}{ch}")
                        for fc in range(FL // 128):
                            nc.tensor.matmul(
                                acc[:], wd[:, fc, 128 * do:128 * (do + 1)],
                                actTc[:, fc, :],
                                start=(fc == 0), stop=(fc == FL // 128 - 1))
                        ev = evh("wdev")
                        nc.scalar.activation(ev[:], acc[:], AF.Copy)
                        nc.sync.dma_start(
                            arin[site][ch]
                            .rearrange("(t p) s -> p t s", p=128)
                            [:, 8 * quarter + 4 * dh + do, :], ev[:])
                nc.gpsimd.collective_compute(
                    "AllReduce", ALU.add,
                    replica_groups=[list(range(NCORES))],
                    ins=[arin[site][ch][:]],
                    outs=[arout[site][ch][:]])

            def lm_chunk(ch, xb, bc_sb, finish_rms=None):
                for vt in range(8):
                    lm = wp_tile(f"lm{ch}{vt}")
                    nc.sync.dma_start(lm[:], lmT[vt])
                    acc = mm_ps(f"lm{vt}{ch}")
                    for c in range(C):
                        nc.tensor.matmul(acc[:],
                                         lm[:, c, :],
                                         xb[:, c, :], start=(c == 0),
                                         stop=(c == C - 1))
                    if vt == 0 and finish_rms is not None:
                        finish_rms()
                    ev = sb.tile([128, 512], dt.float32, tag="evf", bufs=2,
                                 name="lmev")
                    # fold the 1/rms column scale into the evacuation
                    nc.vector.tensor_tensor(ev[:], acc[:], bc_sb[:], ALU.mult)
                    nc.scalar.dma_start(
                        logitsT[:].rearrange("(t p) s -> p t s", p=128)
                        [:, vt, SC * ch:SC * (ch + 1)], ev[:])
                    prev_dep[0] = ev[:]

            # =========================================================
            # main flow: 2-chunk sequence pipeline
            # =========================================================
            def xb_tile(nm, ch):
                return sb.tile([128, C, SC], dt.float16, tag="xb", bufs=2,
                               name=f"{nm}{ch}")

            prev_dep = [None]
            ffn_mid_dep = [None]

            def attn_layer(l, h_src, delta_site, h_dst, site, nm):
                xbs = []
                KT = sb.tile([128, HL, T], dt.float16, tag="KT", bufs=1,
                             name=f"KT{l}")
                Vt = sb.tile([128, HL, NTT, DH], dt.float16, tag="V", bufs=1,
                             name=f"V{l}")
                nc.sync.dma_start(KT[:, :, :M], mkTd[l][:])
                nc.sync.dma_start(Vt[:, :, :NMT, :], mvd[l][:])
                for ch in range(NCH):
                    xb = xb_tile(nm, ch)
                    xbs.append(xb)
                    delta = arout[delta_site][ch][:] if delta_site is not None \
                        else None
                    bc_sb, rs_col, fin = rms_chunk(ch, h_src, delta, h_dst,
                                                   0, xb, nm, "bc",
                                                   order_dep=prev_dep[0])
                    qTc = sb.tile([128, HL, SC], dt.float16, tag="qT", bufs=1,
                                  name=f"qT{l}{ch}")
                    oTc = sb.tile([128, HL, SC], dt.float16, tag="oT", bufs=1,
                                  name=f"oT{l}{ch}")
                    # V halves interleaved with K pairs: the second
                    # V-weight DMA is covered by K-pair compute
                    attn_v_half(l, ch, 0, xb, Vt, rs_col, finish_rms=fin)
                    attn_k_pair(l, ch, 0, xb, KT, bc_sb)
                    attn_v_half(l, ch, 1, xb, Vt, rs_col)
                    attn_k_pair(l, ch, 1, xb, KT, bc_sb)
                    attn_q(l, ch, xb, qTc, bc_sb)
                    attn_S(l, ch, qTc, oTc, KT, Vt)
                    prev_dep[0] = oTc[:, HL - 1, :]
                    attn_wo(l, ch, site, oTc)
                return xbs

            def ffn_layer(l, h_src, delta_site, h_dst, ln_idx, site, nm,
                          xb_in=None):
                for ch in range(NCH):
                    # reuse the attention site's xb tile: it still holds
                    # this chunk's unscaled h (saves a 4MB DRAM re-read)
                    xb = xb_in[ch] if xb_in is not None else xb_tile(nm, ch)
                    rms_chunk(ch, h_src, arout[delta_site][ch][:], h_dst,
                              ln_idx, xb, nm, "scale",
                              order_dep=prev_dep[0],
                              preloaded=(xb_in is not None))
                    actTc = sb.tile([128, FL // 128, SC], dt.float16,
                                    tag="actT", bufs=1, name=f"actT{l}{ch}")
                    ffn_gu(l, ch, xb, actTc)
                    prev_dep[0] = actTc[:, FL // 128 - 1, :]
                    ffn_mid_dep[0] = actTc[:, 2, :]
                    ffn_down(l, ch, site, actTc)

            # memory projections for both layers up front (lean DMA);
            # the 2-chunk pipeline covers the AR windows itself
            mem_proj(0, 0)
            mem_proj(0, 1)
            # warmup collective: absorb first-AR setup cost (placed here
            # so its DMA window doesn't collide with the startup burst)
            wz = wp_tile("warmz")
            nc.vector.memset(wz[:], 0.0)
            nc.sync.dma_start(war_i[:], wz[:])
            nc.gpsimd.collective_compute(
                "AllReduce", ALU.add,
                replica_groups=[list(range(NCORES))],
                ins=[war_i[:]], outs=[war_o[:]])
            mem_proj(1, 0)
            mem_proj(1, 1)
            xbs0 = attn_layer(0, h0T[:], None, None, 0, "x0")
            ffn_layer(0, h0T[:], 0, hdr[0][:], 1, 1, "x1", xb_in=xbs0)
            # layer 1
            xbs1 = attn_layer(1, hdr[0][:], 1, hdr[1][:], 2, "x2")
            ffn_layer(1, hdr[1][:], 2, hdr[2][:], 3, 3, "x3", xb_in=xbs1)
            # final rms + lm head (normw folded into lm weights)
            for ch in range(NCH):
                xb = xb_tile("xf", ch)
                odep = ffn_mid_dep[0] if ch == 0 else prev_dep[0]
                bc_sb, _, fin = rms_chunk(ch, hdr[2][:], arout[3][ch][:],
                                          None, 4, xb, "xf", "bc",
                                          order_dep=odep)
                lm_chunk(ch, xb, bc_sb, finish_rms=fin)

    nc.finalize()
    return nc


_NC_CACHE = {}


def _get_module():
    if "nc" not in _NC_CACHE:
        _NC_CACHE["nc"] = build_module()
    return _NC_CACHE["nc"]


def _rope_tables():
    inv_freq = 1.0 / (ROPE_BASE ** (np.arange(0, DH, 2, dtype=np.float64) / DH))
    ang = np.arange(T, dtype=np.float64)[:, None] * inv_freq[None, :]
    emb = np.concatenate([ang, ang], axis=-1)          # [T, DH]
    return np.cos(emb).astype(np.float32), np.sin(emb).astype(np.float32)


def kernel(input_ids, memory, embed, Wq, Wk, Wv, Wo, Wg, Wu, Wd, Wmk, Wmv,
           ln1, ln2, normw, lm_head):
    input_ids = np.asarray(input_ids)
    f32 = np.float32
    memory = np.asarray(memory, f32)

    nc = _get_module()

    # host prep: embedding gather (pure data movement) + layout transforms
    h0 = np.asarray(embed, f32)[input_ids.reshape(-1)]          # [S, D]
    h0T = np.ascontiguousarray(h0.T).astype(BF)                 # [D, S] fp16

    cos, sin = _rope_tables()
    kcs = np.stack([cos, sin]).transpose(2, 0, 1)               # [128, 2, T]

    rmat = np.zeros((128, 128), f32)
    for d in range(64):
        rmat[d + 64, d] = -1.0
        rmat[d, d + 64] = 1.0

    # multiplicative causal mask (applied to exp(scores))
    tmaskv = np.zeros((128, 896), f32)
    for t in range(128):
        tmaskv[t, 384 + t:] = 1.0

    def bf(x):
        return np.ascontiguousarray(x).astype(BF)

    def swz(wT, nsplit):
        """[Din, n] (Din = c*128) -> [nsplit, 128, c, n/nsplit]."""
        c = wT.shape[0] // 128
        n = wT.shape[1]
        w = wT.reshape(c, 128, n).transpose(1, 0, 2)            # [128, c, n]
        w = w.reshape(128, c, nsplit, n // nsplit).transpose(2, 0, 1, 3)
        return w

    memT = np.stack([swz(memory[l, 0].T, 1)[0] for l in range(L)])

    ln1f = np.asarray(ln1, f32)
    ln2f = np.asarray(ln2, f32)
    normwf = np.asarray(normw, f32)

    in_maps = []
    for i in range(NCORES):
        hs = slice(DL * i, DL * (i + 1))
        fs = slice(FL * i, FL * (i + 1))
        vs = slice(VL * i, VL * (i + 1))
        lnwv = np.stack([np.asarray(ln1, f32)[0], np.asarray(ln2, f32)[0],
                         np.asarray(ln1, f32)[1], np.asarray(ln2, f32)[1],
                         np.asarray(normw, f32)])                # [5, D]
        # ln1 folded into Wq/Wk/Wv (rms scale applied on the output side)
        in_maps.append({
            "h0T": h0T,
            "memT": bf(memT),
            "wqkT": bf(np.stack(
                [np.stack([swz((np.asarray(W, f32)[l][hs]
                                * ln1f[l][None, :]).T, 4)
                           for W in (Wq, Wk)])
                 for l in range(L)])),
            "wvT": bf(np.stack(
                [swz((np.asarray(Wv, f32)[l][hs] * ln1f[l][None, :]).T, 2)
                 for l in range(L)])),
            "wmkT": bf(np.stack([swz(np.asarray(Wmk, f32)[l][hs].T, 4)
                                 for l in range(L)])),
            "wmvT": bf(np.stack([swz(np.asarray(Wmv, f32)[l][hs].T, 2)
                                 for l in range(L)])),
            "woT": bf(np.stack([swz(np.asarray(Wo, f32)[l][:, hs].T, 2)
                                for l in range(L)])),
            "wguT": bf(np.stack(
                [np.stack([swz((np.asarray(W, f32)[l][fs]
                                * ln2f[l][None, :]).T, 8)
                           for W in (Wg, Wu)])
                 for l in range(L)])),
            "wdT": bf(np.stack([swz(np.asarray(Wd, f32)[l][:, fs].T, 4)
                                for l in range(L)])),
            "lmT": bf(swz((np.asarray(lm_head, f32)[vs]
                           * normwf[None, :]).T, 8)),
            "kcs": bf(kcs),
            "rmat": bf(rmat),
            "tmask": bf(tmaskv),
            "lnw": np.ascontiguousarray(
                lnwv.reshape(5, C, 128).transpose(2, 0, 1)),     # [128, 5, C]
        })

    res = run_bass_kernel_spmd(nc, in_maps, core_ids=list(range(NCORES)))
    _NC_CACHE["last_results"] = res

    logits = np.empty((B, S, V), f32)
    for i in range(NCORES):
        logits[0, :, VL * i:VL * (i + 1)] = res.results[i]["logitsT"].T
    return logits


# revision 28
# speedup vs baseline: 1.0056x; 1.0056x over previous
"""Trainium2 Bass kernel for a 2-layer LLaMA-style decoder with per-layer
memory K/V prefix (tokenmix2 Decoder), tensor-parallel over 8 NeuronCores.

Sharding: heads (32 -> 4/core), FFN intermediate (8192 -> 1024/core),
vocab (8192 -> 1024/core).  Two AllReduces per layer site (attention out,
FFN out), one per 512-token sequence chunk ([D, 512] fp16 = 4MB each).

The whole kernel is a 2-stage sequence-chunk pipeline: each chunk's
dependency chain is emitted contiguously, so while chunk c's AllReduce is
in flight the engines run chunk 1-c's compute.

Round-1 perf structure vs the original baseline:
- AR-dependent loads (delta reads from arout) are issued from gpsimd
  (SWDGE) so the sync-engine HWDGE queue never head-of-line blocks on an
  AllReduce-completion semaphore (this froze ALL weight DMA issue for the
  whole AR window in the baseline).
- A 1MB warmup AllReduce at kernel start absorbs first-collective cost.
- rms scaling applied on the OUTPUT side for attention + lm-head sites
  (ln1/normw folded into weights on the host; 1/rms applied via the rope
  raw-evacuation / V-evacuation per-partition scale / logit evacuation).
- rsqrt and softmax reciprocal via exp(-ln(x)...) on the scalar engine
  (replaces the 3.3us DVE iterative reciprocal).
- attention scores pipelined 2 key-tiles ahead of the PV accumulation;
  causal mask is multiplicative 0/1 on exp(scores) in SBUF.
- rope matmuls emitted in 2-head groups behind the next projection
  stream so the PSUM-evacuation latency is hidden; V projection halves
  interleaved with K pairs so the second V-weight DMA is covered.
- all weight DMAs fully contiguous in DRAM; evacuation stores issued
  from the scalar-engine HWDGE queue (separate ring from weight loads).
"""
import sys

sys.path.insert(0, "/opt/trn_rl_repo")

import numpy as np
import ml_dtypes

import concourse.bass as bass
import concourse.mybir as mybir
import concourse.tile as tile
from concourse import bacc
from concourse.bass_utils import run_bass_kernel_spmd

BF = np.float16

# model dims
L, D, H, DH, F, V = 2, 4096, 32, 128, 8192, 8192
B, S, M = 1, 1024, 512
T = M + S                      # 1536 total key positions
EPS = 1e-5
ROPE_BASE = 10000.0
SCALE = float(DH) ** -0.5
LN64 = float(np.log(64.0))

# per-core shards
NCORES = 8
HL = H // NCORES               # 4 local heads
DL = HL * DH                   # 512 local head dims
FL = F // NCORES               # 1024 local ffn
VL = V // NCORES               # 1024 local vocab
C = D // 128                   # 32 contraction tiles
NTT = T // 128                 # 12 key tiles
NMT = M // 128                 # 4 memory key tiles
NST = S // 128                 # 8 query tiles
NCH = 2                        # sequence chunks (512 each)
SC = S // NCH                  # 512

dt = mybir.dt
AF = mybir.ActivationFunctionType
ALU = mybir.AluOpType


def build_module():
    nc = bacc.Bacc("TRN2", target_bir_lowering=False, debug=False,
                   num_devices=NCORES)

    # const APs for activation scale/bias floats
    for v in (EPS, SCALE, 1.0 / D, LN64):
        t = nc.alloc_sbuf_tensor(f"cst_{v}", [128, 1], dt.float32)
        nc.gpsimd.memset(t.ap(), v)
        nc.const_aps.aps[(dt.float32, v)] = t.ap()

    # ---- kernel I/O ----
    h0T = nc.dram_tensor("h0T", [D, S], dt.float16, kind="ExternalInput")
    memT = nc.dram_tensor("memT", [L, 128, C, M], dt.float16, kind="ExternalInput")
    wqkT = nc.dram_tensor("wqkT", [L, 2, 4, 128, C, 128], dt.float16, kind="ExternalInput")
    wvT = nc.dram_tensor("wvT", [L, 2, 128, C, 256], dt.float16, kind="ExternalInput")
    wmkT = nc.dram_tensor("wmkT", [L, 4, 128, C, 128], dt.float16, kind="ExternalInput")
    wmvT = nc.dram_tensor("wmvT", [L, 2, 128, C, 256], dt.float16, kind="ExternalInput")
    woT = nc.dram_tensor("woT", [L, 2, 128, HL, 2048], dt.float16, kind="ExternalInput")
    wguT = nc.dram_tensor("wguT", [L, 2, 8, 128, C, 128], dt.float16, kind="ExternalInput")
    wdT = nc.dram_tensor("wdT", [L, 4, 128, 8, 1024], dt.float16, kind="ExternalInput")
    lmT = nc.dram_tensor("lmT", [8, 128, C, 128], dt.float16, kind="ExternalInput")
    kcs = nc.dram_tensor("kcs", [128, 2, T], dt.float16, kind="ExternalInput")
    rmat_i = nc.dram_tensor("rmat", [128, 128], dt.float16, kind="ExternalInput")
    tmask = nc.dram_tensor("tmask", [128, 896], dt.float16, kind="ExternalInput")
    lnw = nc.dram_tensor("lnw", [128, 5, C], dt.float32, kind="ExternalInput")
    logitsT = nc.dram_tensor("logitsT", [VL, S], dt.float32, kind="ExternalOutput")

    with tile.TileContext(nc) as tc:
        with tc.tile_pool(name="sb", bufs=1) as sb, \
             tc.tile_pool(name="ps", bufs=1, space="PSUM") as ps, \
             tc.tile_pool(name="dr", bufs=1, space="DRAM") as dr:

            # ---- internal DRAM ----
            hdr = [dr.tile([D, S], dt.float16, tag=f"h{i}", bufs=1, name=f"h{i}")
                   for i in range(3)]           # h after resid 1..3
            arin = [[dr.tile([D, SC], dt.float16, tag=f"ai{i}{ch}",
                             bufs=1, name=f"ai{i}{ch}") for ch in range(NCH)]
                    for i in range(2 * L)]
            arout = [[dr.tile([D, SC], dt.float16, tag=f"ao{i}{ch}",
                              bufs=1, addr_space="Shared",
                              name=f"ao{i}{ch}") for ch in range(NCH)]
                     for i in range(2 * L)]
            mkTd = [dr.tile([128, HL, M], dt.float16, tag=f"mk{l}", bufs=1,
                            name=f"mk{l}") for l in range(L)]
            mvd = [dr.tile([128, HL, NMT, DH], dt.float16, tag=f"mv{l}", bufs=1,
                           name=f"mv{l}") for l in range(L)]
            war_i = dr.tile([128, 4096], dt.float16, tag="wri", bufs=1,
                            name="war_i")
            war_o = dr.tile([128, 4096], dt.float16, tag="wro", bufs=1,
                            addr_space="Shared", name="war_o")

            def wp_tile(name):
                return sb.tile([128, C, 128], dt.float16, tag="wp", bufs=3,
                               name=name)

            # ---- global constants in SBUF ----
            kc = sb.tile([128, 2, S], dt.float16, tag="kc", bufs=1, name="kc")
            nc.sync.dma_start(kc[:], kcs[:, :, M:])
            # keys and queries of the S-part sit at the same positions
            qc = kc
            rmat = sb.tile([128, 128], dt.float16, tag="rm", bufs=1, name="rmat")
            nc.sync.dma_start(rmat[:], rmat_i[:])
            mask = sb.tile([128, 896], dt.float16, tag="msk", bufs=1, name="mask")
            nc.sync.dma_start(mask[:], tmask[:])
            lns = sb.tile([128, 5, C], dt.float32, tag="ln", bufs=1, name="lns")
            nc.sync.dma_start(lns[:], lnw[:])
            ones_bf = sb.tile([128, 1], dt.float16, tag="o1", bufs=1, name="ones_bf")
            nc.vector.memset(ones_bf[:], 1.0)
            ones_row = sb.tile([1, 128], dt.float16, tag="o2", bufs=1, name="ones_row")
            nc.vector.memset(ones_row[:], 1.0)
            inv64_row = sb.tile([1, 128], dt.float16, tag="o3", bufs=1, name="inv64_row")
            nc.vector.memset(inv64_row[:], 1.0 / 64.0)

            def mm_ps(name):
                return ps.tile([128, 512], dt.float32, tag="mm", bufs=5, name=name)

            def aux_ps(name):
                return ps.tile([1, 512], dt.float32, tag="aux", bufs=1, name=name)

            def evh(name):
                return sb.tile([128, 512], dt.float16, tag="evh", bufs=4, name=name)

            # ---- rope pieces (split so the rmat matmuls can be emitted
            # behind the next head's projection stream).  If bc_sb is
            # given, the 1/rms row scale is folded into the raw psum
            # evacuation (DVE) -- rope(x*r) == rope(x)*r columnwise. ----
            def rope_start(raw_ps, w=512, bc_sb=None):
                raw_bf = sb.tile([128, 512], dt.float16, tag="rraw", bufs=2,
                                 name="raw_bf")
                if bc_sb is None:
                    nc.scalar.activation(raw_bf[:, :w], raw_ps, AF.Copy)
                else:
                    nc.vector.tensor_tensor(raw_bf[:, :w], raw_ps,
                                            bc_sb[:, :w], ALU.mult)
                return raw_bf

            def rope_mm(raw_bf, w=512):
                r_ps = mm_ps("r_ps")
                nc.tensor.matmul(r_ps[:, :w], rmat[:], raw_bf[:, :w],
                                 start=True, stop=True)
                return r_ps

            def rope_finish(raw_bf, r_ps, cos_ap, sin_ap, out_ap, w=512):
                m1 = sb.tile([128, 512], dt.float16, tag="rt", bufs=2, name="m1")
                nc.gpsimd.tensor_tensor(m1[:, :w], raw_bf[:, :w], cos_ap,
                                        ALU.mult)
                m2 = sb.tile([128, 512], dt.float16, tag="rt2", bufs=2, name="m2")
                nc.vector.tensor_tensor(m2[:, :w], r_ps[:, :w], sin_ap, ALU.mult)
                nc.vector.tensor_tensor(out_ap, m1[:, :w], m2[:, :w], ALU.add)

            # =========================================================
            # memory K/V projections for one layer -> DRAM (M in halves
            # of 256 to halve the mem_sb SBUF footprint)
            # =========================================================
            def mem_proj(l, mh):
                msl = slice(256 * mh, 256 * (mh + 1))
                mem_h = sb.tile([128, C, 256], dt.float16, tag="memh",
                                bufs=1, name=f"mem{l}{mh}")
                nc.sync.dma_start(mem_h[:, :C // 2, :],
                                  memT[l][:, :C // 2, msl])
                nc.sync.dma_start(mem_h[:, C // 2:, :],
                                  memT[l][:, C // 2:, msl])
                # rope tables for memory positions (kc only holds S-part)
                mtab = sb.tile([128, 4, 512], dt.float16, tag="dl", bufs=2,
                               name=f"mtab{l}{mh}")
                nc.sync.dma_start(mtab[:, 0, :], kcs[:, :, msl])
                mcos, msin = mtab[:, 0, 0:256], mtab[:, 0, 256:512]
                def mk_pair(pair):
                    accs = []
                    for dd in range(2):
                        d = 2 * pair + dd
                        wmk = wp_tile(f"wmk{l}{d}{mh}")
                        nc.sync.dma_start(wmk[:], wmkT[l, d])
                        acc = mm_ps(f"mk{l}{d}{mh}")
                        for c in range(C):
                            nc.tensor.matmul(acc[:, :256], wmk[:, c, :],
                                             mem_h[:, c, :], start=(c == 0),
                                             stop=(c == C - 1))
                        accs.append((d, acc[:, :256]))
                    raws = [(d, rope_start(acc, w=256)) for d, acc in accs]
                    rps = [(d, raw, rope_mm(raw, w=256)) for d, raw in raws]
                    for d, raw, rp in rps:
                        mko = sb.tile([128, 256], dt.float16, tag="mko",
                                      bufs=2, name="mko")
                        rope_finish(raw, rp, mcos, msin,
                                    mko[:], w=256)
                        nc.scalar.dma_start(mkTd[l][:, d, msl], mko[:])

                def mv_half(half):
                    wmv = sb.tile([128, C, 256], dt.float16, tag="wv16",
                                  bufs=1, name=f"wmv{l}{mh}{half}")
                    nc.sync.dma_start(wmv[:], wmvT[l, half])
                    for mi in range(2):
                        t = 2 * mh + mi
                        acc = mm_ps(f"mv{l}{mh}{half}{mi}")
                        for c in range(C):
                            nc.tensor.matmul(
                                acc[:, :256],
                                mem_h[:, c, 128 * mi:128 * (mi + 1)],
                                wmv[:, c, :], start=(c == 0),
                                stop=(c == C - 1))
                        mvo = sb.tile([128, 256], dt.float16, tag="mvo",
                                      bufs=2, name="mvo")
                        nc.scalar.activation(mvo[:], acc[:, :256], AF.Copy)
                        nc.scalar.dma_start(
                            mvd[l][:, 2 * half:2 * half + 2, t, :], mvo[:])

                # interleave so the second wmv DMA hides behind mk compute
                mk_pair(0)
                mv_half(0)
                mk_pair(1)
                mv_half(1)

            # =========================================================
            # rms for ONE chunk.
            # mode "scale": scale xb in place (ffn sites).
            # mode "bc": no input scaling -- returns (bc_sb fp16 [128,SC]
            #   row-broadcast of 1/rms, rs_col f32 [128,4] column form).
            #   ln / final-norm weights are folded into the weights on
            #   the host for these sites.
            # =========================================================
            def rms_chunk(ch, h_src, delta, h_dst, ln_idx, xb, name, mode,
                          order_dep=None, preloaded=False):
                ssq = aux_ps(f"ssq_{name}{ch}")
                if not preloaded:
                    hv = h_src.rearrange("(c p) s -> p c s", p=128)
                    # prefetch the full h chunk first (not AR-gated)
                    for cq in range(C // 2):
                        csl = slice(2 * cq, 2 * cq + 2)
                        nc.sync.dma_start(xb[:, csl, :],
                                          hv[:, csl, SC * ch:SC * (ch + 1)])
                for cq in range(C // 4):
                    csl = slice(4 * cq, 4 * cq + 4)
                    if delta is not None:
                        dtl = sb.tile([128, 4, 512], dt.float16, tag="dl",
                                      bufs=2, name="dtl")
                        if order_dep is not None and cq < 2:
                            # dummy write that READS the previous chunk's
                            # last vector output: forces the scheduler to
                            # place this whole AR-gated chain after all of
                            # the previous chunk's engine work, so the
                            # in-order queues never head-of-line block on
                            # the AllReduce semaphore.
                            nc.vector.tensor_tensor(dtl[:, 0, :], order_dep,
                                                    order_dep, ALU.mult)
                        # gpsimd (SWDGE) so the AR-completion wait never
                        # blocks the sync HWDGE queue
                        nc.gpsimd.dma_start(
                            dtl[:],
                            delta.rearrange("(c p) s -> p c s", p=128)[:, csl, :])
                        for ci in range(4):
                            nc.vector.tensor_tensor(xb[:, 4 * cq + ci, :],
                                                    xb[:, 4 * cq + ci, :],
                                                    dtl[:, ci, :], ALU.add)
                        if h_dst is not None:
                            nc.sync.dma_start(
                                h_dst.rearrange("(c p) s -> p c s", p=128)
                                [:, csl, SC * ch:SC * (ch + 1)],
                                xb[:, csl, :])
                    for ci in range(4):
                        c = 4 * cq + ci
                        hsq = sb.tile([128, 512], dt.float16, tag="hsq",
                                      bufs=2, name="hsq")
                        nc.vector.tensor_tensor(hsq[:], xb[:, c, :],
                                                xb[:, c, :], ALU.mult)
                        nc.tensor.matmul(ssq[:], ones_bf[:], hsq[:],
                                         start=(c == 0), stop=(c == C - 1))
                # 1/rms row: exp(-0.5 * ln(ssq/D + eps)) on ACT (fast;
                # avoids the 3.3us DVE iterative reciprocal)
                def emit_row():
                    lnr = sb.tile([1, 512], dt.float32, tag="row", bufs=1,
                                  name="lnr")
                    nc.scalar.activation(lnr[:], ssq[:], AF.Ln, bias=EPS,
                                         scale=1.0 / D)
                    rs = sb.tile([1, 512], dt.float16, tag="row2", bufs=2,
                                 name="rs")
                    nc.scalar.activation(rs[:], lnr[:], AF.Exp, scale=-0.5)
                    bc_ps = ps.tile([128, 512], dt.float32, tag="bc", bufs=1,
                                    name="bc")
                    nc.tensor.matmul(bc_ps[:], ones_row[:], rs[:], start=True,
                                     stop=True)
                    nc.scalar.activation(bc_sb[:], bc_ps[:], AF.Copy)
                    return rs
                if mode == "scale":
                    # ln2 is folded into Wg/Wu on the host, so the rescale
                    # is a plain 2-operand multiply (fast DVE mode)
                    bc_sb = sb.tile([128, 512], dt.float16, tag="bcb", bufs=2,
                                    name="bcs16")
                    emit_row()
                    for c in range(C):
                        nc.vector.tensor_tensor(
                            xb[:, c, :], xb[:, c, :], bc_sb[:], ALU.mult)
                    return None, None, None
                bc_sb = sb.tile([128, 512], dt.float16, tag="bcb", bufs=2,
                                name="bc_sb")
                rs_col = sb.tile([128, 4], dt.float32, tag="rscs", bufs=2,
                                 name="rs_col")

                def finish():
                    # deferred: emitted behind the first projection block
                    # so the ACT row latency never stalls the PE
                    rs = emit_row()
                    rsc = ps.tile([128, 4], dt.float32, tag="rsc", bufs=1,
                                  name="rsc")
                    for st in range(4):
                        nc.tensor.matmul(rsc[:, st:st + 1],
                                         rs[:, 128 * st:128 * (st + 1)],
                                         ones_row[:, 0:1], start=True,
                                         stop=True)
                    nc.scalar.activation(rs_col[:], rsc[:], AF.Copy)
                return bc_sb, rs_col, finish

            # =========================================================
            # attention pieces (KT/Vt persist for the layer)
            # =========================================================
            def attn_v_half(l, ch, half, xb, Vt, rs_col, finish_rms=None):
                wv = sb.tile([128, C, 256], dt.float16, tag="wv16",
                             bufs=1, name=f"wv{l}{ch}{half}")
                nc.sync.dma_start(wv[:], wvT[l, half])
                for sti in range(4):
                    st = 4 * ch + sti
                    acc = mm_ps(f"v{l}{half}{st}")
                    for c in range(C):
                        nc.tensor.matmul(
                            acc[:, :256],
                            xb[:, c, 128 * sti:128 * (sti + 1)],
                            wv[:, c, :], start=(c == 0), stop=(c == C - 1))
                    if sti == 0 and finish_rms is not None:
                        finish_rms()
                    # per-key 1/rms via per-partition ACT scale
                    nc.scalar.activation(Vt[:, 2 * half:2 * half + 2,
                                            NMT + st, :],
                                         acc[:, :256], AF.Copy,
                                         scale=rs_col[:, sti:sti + 1])

            def attn_k_pair(l, ch, pair, xb, KT, bc_sb):
                accs = []
                for dd in range(2):
                    d = 2 * pair + dd
                    wk = wp_tile(f"wk{l}{ch}{d}")
                    nc.sync.dma_start(wk[:], wqkT[l, 1, d])
                    acc = mm_ps(f"k{l}{d}{ch}")
                    for c in range(C):
                        nc.tensor.matmul(
                            acc[:], wk[:, c, :],
                            xb[:, c, :], start=(c == 0), stop=(c == C - 1))
                    accs.append((d, acc))
                raws = [(d, rope_start(acc, bc_sb=bc_sb)) for d, acc in accs]
                rps = [(d, raw, rope_mm(raw)) for d, raw in raws]
                for d, raw, rp in rps:
                    rope_finish(raw, rp,
                                kc[:, 0, SC * ch:SC * (ch + 1)],
                                kc[:, 1, SC * ch:SC * (ch + 1)],
                                KT[:, d, M + SC * ch:M + SC * (ch + 1)])

            def attn_q(l, ch, xb, qTc, bc_sb):
                for pair in range(2):
                    accs = []
                    for hh in range(2):
                        h = 2 * pair + hh
                        wqh = wp_tile(f"wq{l}{ch}{h}")
                        nc.sync.dma_start(wqh[:], wqkT[l, 0, h])
                        acc = mm_ps(f"q{l}{h}{ch}")
                        for c in range(C):
                            nc.tensor.matmul(
                                acc[:], wqh[:, c, :],
                                xb[:, c, :], start=(c == 0), stop=(c == C - 1))
                        accs.append((h, acc))
                    raws = [(h, rope_start(acc, bc_sb=bc_sb)) for h, acc in accs]
                    rps = [(h, raw, rope_mm(raw)) for h, raw in raws]
                    for h, raw, rp in rps:
                        rope_finish(raw, rp,
                                    qc[:, 0, SC * ch:SC * (ch + 1)],
                                    qc[:, 1, SC * ch:SC * (ch + 1)],
                                    qTc[:, h, :])

            def attn_S(l, sb_i, qTc, oTc, KT, Vt):
                ntt = NMT + 4 * (sb_i + 1)
                LA = 3
                pending = [None]

                def normalize_act():
                    # ACT row chain emitted right behind the head's last
                    # s-matmul, ahead of the next head's exp stream
                    h, o_ps, s_ps = pending[0]
                    lnr = sb.tile([1, 512], dt.float32, tag="row", bufs=1,
                                  name="lnr_s")
                    nc.scalar.activation(lnr[:], s_ps[:], AF.Ln)
                    rr = sb.tile([1, 512], dt.float16, tag="row2", bufs=2,
                                 name="rr")
                    nc.scalar.activation(rr[:], lnr[:], AF.Exp, scale=-1.0,
                                         bias=LN64)
                    pending[0] = (h, o_ps, rr)

                def normalize_mm():
                    h, o_ps, rr = pending[0]
                    bcp = ps.tile([128, 512], dt.float32, tag="bc", bufs=1,
                                  name="bca")
                    nc.tensor.matmul(bcp[:], inv64_row[:], rr[:],
                                     start=True, stop=True)
                    bcs = sb.tile([128, 512], dt.float32, tag="bcs",
                                  bufs=1, name="bcs")
                    nc.vector.tensor_copy(bcs[:], bcp[:])
                    nc.vector.tensor_tensor(oTc[:, h, :], o_ps[:], bcs[:],
                                            ALU.mult)
                    pending[0] = None

                for h in range(HL):
                    s_ps = aux_ps(f"s{l}{h}{sb_i}")
                    pts = {}

                    def emit_sc(tt, h=h):
                        sc_ps = mm_ps(f"sc{l}{h}{sb_i}{tt}")
                        nc.tensor.matmul(sc_ps[:],
                                         KT[:, h, 128 * tt:128 * (tt + 1)],
                                         qTc[:, h, :], start=True, stop=True)
                        pt = sb.tile([128, 512], dt.float16, tag="pt",
                                     bufs=4, name="pt")
                        nc.scalar.activation(pt[:], sc_ps[:], AF.Exp,
                                             scale=SCALE)
                        dtile = tt - ntt + 4      # >= 0 -> diagonal tile
                        if dtile >= 0:
                            off = 384 - 128 * dtile
                            nc.vector.tensor_tensor(
                                pt[:], pt[:],
                                mask[:, off:off + 512], ALU.mult)
                        pts[tt] = pt

                    if pending[0] is not None:
                        normalize_act()
                    for tt in range(min(LA, ntt)):
                        emit_sc(tt)
                    o_ps = mm_ps(f"o{l}{h}{sb_i}")
                    for tt in range(ntt):
                        pt = pts.pop(tt)
                        nc.tensor.matmul(o_ps[:], Vt[:, h, tt, :], pt[:],
                                         start=(tt == 0),
                                         stop=(tt == ntt - 1))
                        nc.tensor.matmul(s_ps[:], ones_bf[:], pt[:],
                                         start=(tt == 0),
                                         stop=(tt == ntt - 1))
                        if tt + LA < ntt:
                            emit_sc(tt + LA)
                        # previous head's broadcast-mm lands two tt-groups
                        # into this head: full cover for the ACT row chain
                        if tt == 1 and pending[0] is not None:
                            normalize_mm()
                    pending[0] = (h, o_ps, s_ps)
                normalize_act()
                normalize_mm()

            def attn_wo(l, ch, site, oTc):
                for half in range(2):
                  for doh in range(2):
                    wo = sb.tile([128, HL, 1024], dt.float16, tag="wp", bufs=3,
                                 name=f"wo{l}{ch}{half}{doh}")
                    nc.sync.dma_start(
                        wo[:], woT[l, half][:, :, 1024 * doh:1024 * (doh + 1)])
                    for do in range(8):
                        acc = mm_ps(f"wo{l}{half}{doh}{do}{ch}")
                        for hh in range(HL):
                            nc.tensor.matmul(
                                acc[:], wo[:, hh, 128 * do:128 * (do + 1)],
                                oTc[:, hh, :],
                                start=(hh == 0), stop=(hh == HL - 1))
                        ev = evh("woev")
                        # alternate evacuation engines so a single queue
                        # never paces the PSUM free rate
                        if do % 2 == 0:
                            nc.scalar.activation(ev[:], acc[:], AF.Copy)
                        else:
                            nc.vector.tensor_copy(ev[:], acc[:])
                        nc.sync.dma_start(
                            arin[site][ch]
                            .rearrange("(t p) s -> p t s", p=128)
                            [:, 16 * half + 8 * doh + do, :], ev[:])
                nc.gpsimd.collective_compute(
                    "AllReduce", ALU.add,
                    replica_groups=[list(range(NCORES))],
                    ins=[arin[site][ch][:]],
                    outs=[arout[site][ch][:]])

            # =========================================================
            # FFN pieces
            # =========================================================
            def ffn_gu(l, ch, xb, actTc):
                for fe in range(FL // 128):
                    wg = wp_tile(f"wg{l}{ch}{fe}")
                    nc.sync.dma_start(wg[:], wguT[l, 0, fe])
                    gs = sb.tile([128, 512], dt.float16, tag="gs", bufs=2,
                                 name="gs")
                    acc = mm_ps(f"g{l}{fe}{ch}")
                    for c in range(C):
                        nc.tensor.matmul(acc[:], wg[:, c, :],
                                         xb[:, c, :], start=(c == 0),
                                         stop=(c == C - 1))
                    nc.scalar.activation(gs[:], acc[:], AF.Silu)
                    wu = wp_tile(f"wu{l}{ch}{fe}")
                    nc.sync.dma_start(wu[:], wguT[l, 1, fe])
                    acc2 = mm_ps(f"u{l}{fe}{ch}")
                    for c in range(C):
                        nc.tensor.matmul(acc2[:], wu[:, c, :],
                                         xb[:, c, :], start=(c == 0),
                                         stop=(c == C - 1))
                    nc.vector.tensor_tensor(actTc[:, fe, :], acc2[:],
                                            gs[:], ALU.mult)

            def ffn_down(l, ch, site, actTc):
                for quarter in range(4):
                  for dh in range(2):
                    wd = sb.tile([128, FL // 128, 512], dt.float16, tag="wp",
                                 bufs=3, name=f"wd{l}{ch}{quarter}{dh}")
                    nc.sync.dma_start(
                        wd[:], wdT[l, quarter][:, :, 512 * dh:512 * (dh + 1)])
                    for do in range(4):
                        acc = mm_ps(f"wd{l}{quarter}{dh}{do}{ch}")
                        for fc in range(FL // 128):
                            nc.tensor.matmul(
                                acc[:], wd[:, fc, 128 * do:128 * (do + 1)],
                                actTc[:, fc, :],
                                start=(fc == 0), stop=(fc == FL // 128 - 1))
                        ev = evh("wdev")
                        nc.scalar.activation(ev[:], acc[:], AF.Copy)
                        nc.sync.dma_start(
                            arin[site][ch]
                            .rearrange("(t p) s -> p t s", p=128)
                            [:, 8 * quarter + 4 * dh + do, :], ev[:])
                nc.gpsimd.collective_compute(
                    "AllReduce", ALU.add,
                    replica_groups=[list(range(NCORES))],
                    ins=[arin[site][ch][:]],
                    outs=[arout[site][ch][:]])

            def lm_chunk(ch, xb, bc_sb, finish_rms=None):
                for vt in range(8):
                    lm = wp_tile(f"lm{ch}{vt}")
                    nc.sync.dma_start(lm[:], lmT[vt])
                    acc = mm_ps(f"lm{vt}{ch}")
                    for c in range(C):
                        nc.tensor.matmul(acc[:],
                                         lm[:, c, :],
                                         xb[:, c, :], start=(c == 0),
                                         stop=(c == C - 1))
                    if vt == 0 and finish_rms is not None:
                        finish_rms()
                    ev = sb.tile([128, 512], dt.float32, tag="evf", bufs=2,
                                 name="lmev")
                    # fold the 1/rms column scale into the evacuation
                    nc.vector.tensor_tensor(ev[:], acc[:], bc_sb[:], ALU.mult)
                    nc.scalar.dma_start(
                        logitsT[:].rearrange("(t p) s -> p t s", p=128)
                        [:, vt, SC * ch:SC * (ch + 1)], ev[:])
                    prev_dep[0] = ev[:]

            # =========================================================
            # main flow: 2-chunk sequence pipeline
            # =========================================================
            def xb_tile(nm, ch):
                return sb.tile([128, C, SC], dt.float16, tag="xb", bufs=2,
                               name=f"{nm}{ch}")

            prev_dep = [None]
            ffn_mid_dep = [None]

            def attn_layer(l, h_src, delta_site, h_dst, site, nm):
                xbs = []
                KT = sb.tile([128, HL, T], dt.float16, tag="KT", bufs=1,
                             name=f"KT{l}")
                Vt = sb.tile([128, HL, NTT, DH], dt.float16, tag="V", bufs=1,
                             name=f"V{l}")
                nc.sync.dma_start(KT[:, :, :M], mkTd[l][:])
                nc.sync.dma_start(Vt[:, :, :NMT, :], mvd[l][:])
                for ch in range(NCH):
                    xb = xb_tile(nm, ch)
                    xbs.append(xb)
                    delta = arout[delta_site][ch][:] if delta_site is not None \
                        else None
                    bc_sb, rs_col, fin = rms_chunk(ch, h_src, delta, h_dst,
                                                   0, xb, nm, "bc",
                                                   order_dep=prev_dep[0])
                    qTc = sb.tile([128, HL, SC], dt.float16, tag="qT", bufs=1,
                                  name=f"qT{l}{ch}")
                    oTc = sb.tile([128, HL, SC], dt.float16, tag="oT", bufs=1,
                                  name=f"oT{l}{ch}")
                    # V halves interleaved with K pairs: the second
                    # V-weight DMA is covered by K-pair compute
                    attn_v_half(l, ch, 0, xb, Vt, rs_col, finish_rms=fin)
                    attn_k_pair(l, ch, 0, xb, KT, bc_sb)
                    attn_v_half(l, ch, 1, xb, Vt, rs_col)
                    attn_k_pair(l, ch, 1, xb, KT, bc_sb)
                    attn_q(l, ch, xb, qTc, bc_sb)
                    attn_S(l, ch, qTc, oTc, KT, Vt)
                    prev_dep[0] = oTc[:, HL - 1, :]
                    attn_wo(l, ch, site, oTc)
                return xbs

            def ffn_layer(l, h_src, delta_site, h_dst, ln_idx, site, nm,
                          xb_in=None):
                for ch in range(NCH):
                    # reuse the attention site's xb tile: it still holds
                    # this chunk's unscaled h (saves a 4MB DRAM re-read)
                    xb = xb_in[ch] if xb_in is not None else xb_tile(nm, ch)
                    rms_chunk(ch, h_src, arout[delta_site][ch][:], h_dst,
                              ln_idx, xb, nm, "scale",
                              order_dep=prev_dep[0],
                              preloaded=(xb_in is not None))
                    actTc = sb.tile([128, FL // 128, SC], dt.float16,
                                    tag="actT", bufs=1, name=f"actT{l}{ch}")
                    ffn_gu(l, ch, xb, actTc)
                    prev_dep[0] = actTc[:, FL // 128 - 1, :]
                    ffn_mid_dep[0] = actTc[:, 2, :]
                    ffn_down(l, ch, site, actTc)

            # memory projections for both layers up front (lean DMA);
            # the 2-chunk pipeline covers the AR windows itself
            mem_proj(0, 0)
            mem_proj(0, 1)
            # warmup collective: absorb first-AR setup cost (placed here
            # so its DMA window doesn't collide with the startup burst)
            wz = wp_tile("warmz")
            nc.vector.memset(wz[:], 0.0)
            nc.sync.dma_start(war_i[:], wz[:])
            nc.gpsimd.collective_compute(
                "AllReduce", ALU.add,
                replica_groups=[list(range(NCORES))],
                ins=[war_i[:]], outs=[war_o[:]])
            mem_proj(1, 0)
            mem_proj(1, 1)
            xbs0 = attn_layer(0, h0T[:], None, None, 0, "x0")
            ffn_layer(0, h0T[:], 0, hdr[0][:], 1, 1, "x1", xb_in=xbs0)
            # layer 1
            xbs1 = attn_layer(1, hdr[0][:], 1, hdr[1][:], 2, "x2")
            ffn_layer(1, hdr[1][:], 2, hdr[2][:], 3, 3, "x3", xb_in=xbs1)
            # final rms + lm head (normw folded into lm weights)
            for ch in range(NCH):
                xb = xb_tile("xf", ch)
                odep = ffn_mid_dep[0] if ch == 0 else prev_dep[0]
                bc_sb, _, fin = rms_chunk(ch, hdr[2][:], arout[3][ch][:],
                                          None, 4, xb, "xf", "bc",
                                          order_dep=odep)
                lm_chunk(ch, xb, bc_sb, finish_rms=fin)

    nc.finalize()
    return nc


_NC_CACHE = {}


def _get_module():
    if "nc" not in _NC_CACHE:
        _NC_CACHE["nc"] = build_module()
    return _NC_CACHE["nc"]


def _rope_tables():
    inv_freq = 1.0 / (ROPE_BASE ** (np.arange(0, DH, 2, dtype=np.float64) / DH))
    ang = np.arange(T, dtype=np.float64)[:, None] * inv_freq[None, :]
    emb = np.concatenate([ang, ang], axis=-1)          # [T, DH]
    return np.cos(emb).astype(np.float32), np.sin(emb).astype(np.float32)


def kernel(input_ids, memory, embed, Wq, Wk, Wv, Wo, Wg, Wu, Wd, Wmk, Wmv,
           ln1, ln2, normw, lm_head):
    input_ids = np.asarray(input_ids)
    f32 = np.float32
    memory = np.asarray(memory, f32)

    nc = _get_module()

    # host prep: embedding gather (pure data movement) + layout transforms
    h0 = np.asarray(embed, f32)[input_ids.reshape(-1)]          # [S, D]
    h0T = np.ascontiguousarray(h0.T).astype(BF)                 # [D, S] fp16

    cos, sin = _rope_tables()
    kcs = np.stack([cos, sin]).transpose(2, 0, 1)               # [128, 2, T]

    rmat = np.zeros((128, 128), f32)
    for d in range(64):
        rmat[d + 64, d] = -1.0
        rmat[d, d + 64] = 1.0

    # multiplicative causal mask (applied to exp(scores))
    tmaskv = np.zeros((128, 896), f32)
    for t in range(128):
        tmaskv[t, 384 + t:] = 1.0

    def bf(x):
        return np.ascontiguousarray(x).astype(BF)

    def swz(wT, nsplit):
        """[Din, n] (Din = c*128) -> [nsplit, 128, c, n/nsplit]."""
        c = wT.shape[0] // 128
        n = wT.shape[1]
        w = wT.reshape(c, 128, n).transpose(1, 0, 2)            # [128, c, n]
        w = w.reshape(128, c, nsplit, n // nsplit).transpose(2, 0, 1, 3)
        return w

    memT = np.stack([swz(memory[l, 0].T, 1)[0] for l in range(L)])

    ln1f = np.asarray(ln1, f32)
    ln2f = np.asarray(ln2, f32)
    normwf = np.asarray(normw, f32)

    in_maps = []
    for i in range(NCORES):
        hs = slice(DL * i, DL * (i + 1))
        fs = slice(FL * i, FL * (i + 1))
        vs = slice(VL * i, VL * (i + 1))
        lnwv = np.stack([np.asarray(ln1, f32)[0], np.asarray(ln2, f32)[0],
                         np.asarray(ln1, f32)[1], np.asarray(ln2, f32)[1],
                         np.asarray(normw, f32)])                # [5, D]
        # ln1 folded into Wq/Wk/Wv (rms scale applied on the output side)
        in_maps.append({
            "h0T": h0T,
            "memT": bf(memT),
            "wqkT": bf(np.stack(
                [np.stack([swz((np.asarray(W, f32)[l][hs]
                                * ln1f[l][None, :]).T, 4)
                           for W in (Wq, Wk)])
                 for l in range(L)])),
            "wvT": bf(np.stack(
                [swz((np.asarray(Wv, f32)[l][hs] * ln1f[l][None, :]).T, 2)
                 for l in range(L)])),
            "wmkT": bf(np.stack([swz(np.asarray(Wmk, f32)[l][hs].T, 4)
                                 for l in range(L)])),
            "wmvT": bf(np.stack([swz(np.asarray(Wmv, f32)[l][hs].T, 2)
                                 for l in range(L)])),
            "woT": bf(np.stack([swz(np.asarray(Wo, f32)[l][:, hs].T, 2)
                                for l in range(L)])),
            "wguT": bf(np.stack(
                [np.stack([swz((np.asarray(W, f32)[l][fs]
                                * ln2f[l][None, :]).T, 8)
                           for W in (Wg, Wu)])
                 for l in range(L)])),
            "wdT": bf(np.stack([swz(np.asarray(Wd, f32)[l][:, fs].T, 4)
                                for l in range(L)])),
            "lmT": bf(swz((np.asarray(lm_head, f32)[vs]
                           * normwf[None, :]).T, 8)),
            "kcs": bf(kcs),
            "rmat": bf(rmat),
            "tmask": bf(tmaskv),
            "lnw": np.ascontiguousarray(
                lnwv.reshape(5, C, 128).transpose(2, 0, 1)),     # [128, 5, C]
        })

    res = run_bass_kernel_spmd(nc, in_maps, core_ids=list(range(NCORES)))
    _NC_CACHE["last_results"] = res

    logits = np.empty((B, S, V), f32)
    for i in range(NCORES):
        logits[0, :, VL * i:VL * (i + 1)] = res.results[i]["logitsT"].T
    return logits
